# revision 25
# baseline (speedup 1.0000x reference)
"""AdaConv2d (per-sample masked 3x3 conv) on 8 TRN2 NeuronCores.

Strategy (data-parallel, per sharding hint):
  - 64 samples sharded 8-per-core; kernel_base/kernel_mask replicated.
  - Host ships, per sample, a [128, 114*114] bf16 buffer: partitions
    0-63 hold the zero-padded image (one input channel per partition),
    partitions 64-127 hold the same image shifted down one padded row.
  - Conv = 6 accumulated matmul passes per 4-row output block:
    3 "big" passes contract K=128 = (tap(0,dx) on the top half,
    tap(1,dx) on the shifted bottom half) and 3 "small" K=64 passes
    cover tap(2,dx). TRN2 streams ~2 rhs columns/cycle across the two
    64-column PE groups, so two blocks (even->PSUM parts 0-63,
    odd->parts 64-127) run concurrently.
  - Per-sample kernels = kernel_base * kernel_mask[label] computed on
    device (one broadcast tensor_tensor per sample, cast to bf16); the
    host only does layout/padding transposes and the label gather of
    mask rows.
"""
import numpy as np
import ml_dtypes

import concourse.bass as bass  # noqa: F401  (registers engines)
import concourse.tile as tile
from concourse.tile_rust import add_dep_helper
from concourse import bacc, mybir
from concourse.bass_utils import run_bass_kernel_spmd

NCORES = 8
SPC = 8            # samples per core
H = W = 112
IC = OC = 64
ND = 4             # demographic groups
PW = H + 2         # padded width/height
PHW = PW * PW
RB = 4             # output rows per matmul block
N = RB * W         # 448 columns per matmul (one PSUM bank)
BLOCKS = H // RB   # 28 blocks per sample
ROUNDS = BLOCKS // 2
NPASS = 6          # 3 big (K=128) + 3 small (K=64)
FUSE_EPOCH = 9
F32 = mybir.dt.float32
BF16 = mybir.dt.bfloat16

# weight column layout: j=0..2 big pass dx=j (top=tap(0,dx),
# bottom=tap(1,dx)); j=3..5 small pass dx=j-3 (top=tap(2,dx))
TAP_TOP = [(0, 0), (0, 1), (0, 2), (2, 0), (2, 1), (2, 2)]
TAP_BOT = [(1, 0), (1, 1), (1, 2), (2, 0), (2, 1), (2, 2)]

_CACHE = {}


def _build():
    nc = bacc.Bacc("TRN2", target_bir_lowering=False, debug=False,
                   num_devices=NCORES)
    xs = nc.dram_tensor("xs", [SPC, 128, PHW], BF16,
                        kind="ExternalInput").ap()
    bT = nc.dram_tensor("bT", [128, NPASS * OC], F32,
                        kind="ExternalInput").ap()
    msel = nc.dram_tensor("msel", [128, SPC * NPASS], F32,
                          kind="ExternalInput").ap()
    out = nc.dram_tensor("out", [SPC, OC, H, W], BF16,
                         kind="ExternalOutput").ap()

    # [sample, round, blk, oc, rb*w]
    ov = out.rearrange("b oc (r blk rb) w -> b r blk oc (rb w)",
                       blk=2, rb=RB)

    with tile.TileContext(nc) as tc:
        with (
            tc.tile_pool(name="const", bufs=1) as constp,
            tc.tile_pool(name="xp", bufs=3) as xp,
            tc.tile_pool(name="wp", bufs=2) as wp,
            tc.tile_pool(name="stage", bufs=14) as stp,
            tc.tile_pool(name="psum", bufs=4, space="PSUM") as pp,
        ):
            bT_t = constp.tile([128, NPASS * OC], F32, name="bT_t",
                               tag="bT_t")
            nc.scalar.dma_start(bT_t[:], bT[:])
            msel_t = constp.tile([128, SPC * NPASS], F32, name="msel_t",
                                 tag="msel_t")
            nc.scalar.dma_start(msel_t[:], msel[:])
            b3 = bT_t.rearrange("p (j oc) -> p j oc", oc=OC)

            prev_round_mm = None
            for s in range(SPC):
                xt = xp.tile([128, PHW], BF16, name="xt", tag="xt")
                # chunked loads, paced against the previous sample's
                # compute so input bursts never starve the output DMAs
                NCH = 8
                for q in range(NCH):
                    qs = (PHW // NCH) * q
                    qe = PHW if q == NCH - 1 else (PHW // NCH) * (q + 1)
                    d = nc.scalar.dma_start(xt[:, qs:qe], xs[s][:, qs:qe])
                    if prev_round_mm is not None and q > 0:
                        gate = prev_round_mm[(q * ROUNDS) // NCH]
                        add_dep_helper(gate.ins, d.ins, sync=True,
                                       reason="pace x load")
                round_mm = []
                x3 = xt.rearrange("p (r c) -> p r c", c=PW)

                wt = wp.tile([128, NPASS * OC], BF16, name="wt", tag="wt")
                w3 = wt.rearrange("p (j oc) -> p j oc", oc=OC)
                m3 = (msel_t[:, s * NPASS:(s + 1) * NPASS]
                      .unsqueeze(-1).broadcast_to([128, NPASS, OC]))
                nc.vector.tensor_tensor(w3[:], b3[:], m3,
                                        op=mybir.AluOpType.mult)

                for rnd in range(ROUNDS):
                    psE = pp.tile([128, N], F32, name="psE", tag="psE")
                    psO = pp.tile([128, N], F32, name="psO", tag="psO")
                    ps = (psE, psO)
                    for j in range(NPASS):
                        dx = j % 3
                        first, last = (j == 0), (j == NPASS - 1)
                        for blk in range(2):
                            r0 = (rnd * 2 + blk) * RB
                            pc = blk * 64
                            if j < 3:   # big pass: K=128, taps (0/1, dx)
                                rhs = x3[:, r0:r0 + RB, dx:dx + W]
                                lhsT = w3[:, j, :]
                            else:       # small pass: K=64, tap (2, dx)
                                rhs = x3[0:64, r0 + 2:r0 + 2 + RB,
                                         dx:dx + W]
                                lhsT = w3[0:64, j, :]
                            mm = nc.tensor.matmul(ps[blk][pc:pc + 64, :],
                                                  lhsT, rhs,
                                                  start=first, stop=last)
                    round_mm.append(mm)

                    st = stp.tile([128, N], BF16, name="st", tag="st")
                    nc.vector.tensor_copy(st[0:64, :], psE[0:64, :])
                    nc.scalar.copy(st[64:128, :], psO[64:128, :])
                    # per-block DMAs: 64-partition outer dim spreads the
                    # transfer across all SDMA engines
                    nc.sync.dma_start(ov[s, rnd, 0], st[0:64, :])
                    nc.sync.dma_start(ov[s, rnd, 1], st[64:128, :])
                prev_round_mm = round_mm

    nc.compile()
    return nc


def get_nc():
    if "nc" not in _CACHE:
        _CACHE["nc"] = _build()
    return _CACHE["nc"]


def make_in_maps(x, kernel_base, kernel_mask, demog_label, epoch):
    kb = np.asarray(kernel_base, dtype=np.float32)
    km = np.asarray(kernel_mask, dtype=np.float32)
    labels = np.asarray(demog_label).astype(np.int64)
    if int(np.asarray(epoch)) >= FUSE_EPOCH:
        labels = np.zeros_like(labels)

    B = labels.shape[0]
    # padded bf16 image + one-row-down shifted copy (layout only)
    xb = np.asarray(x, dtype=np.float32).astype(ml_dtypes.bfloat16)
    xpad = np.zeros((B, IC, PW, PW), dtype=ml_dtypes.bfloat16)
    xpad[:, :, 1:H + 1, 1:W + 1] = xb
    flat = xpad.reshape(B, IC, PHW)
    xfull = np.zeros((B, 128, PHW), dtype=ml_dtypes.bfloat16)
    xfull[:, 0:IC, :] = flat
    xfull[:, IC:, :PHW - PW] = flat[:, :, PW:]

    # bT2[p, j, oc]: top = base taps TAP_TOP, bottom = TAP_BOT (0 pad)
    kb9 = kb.reshape(OC, IC, 9)            # tap index = 3*dy + dx
    bT2 = np.zeros((128, NPASS, OC), dtype=np.float32)
    km9 = km.reshape(ND, IC, 9)
    for j, (dy, dx) in enumerate(TAP_TOP):
        bT2[0:IC, j, :] = kb9[:, :, 3 * dy + dx].T
    for j, tap in enumerate(TAP_BOT):
        if tap is not None:
            dy, dx = tap
            bT2[IC:, j, :] = kb9[:, :, 3 * dy + dx].T
    bT2 = bT2.reshape(128, NPASS * OC)

    in_maps = []
    for c in range(NCORES):
        lab = labels[c * SPC:(c + 1) * SPC]
        msel = np.zeros((128, SPC * NPASS), dtype=np.float32)
        for s in range(SPC):
            for j, (dy, dx) in enumerate(TAP_TOP):
                msel[0:IC, s * NPASS + j] = km9[lab[s], :, 3 * dy + dx]
            for j, tap in enumerate(TAP_BOT):
                if tap is not None:
                    dy, dx = tap
                    msel[IC:, s * NPASS + j] = km9[lab[s], :, 3 * dy + dx]
        in_maps.append({
            "xs": np.ascontiguousarray(xfull[c * SPC:(c + 1) * SPC]),
            "bT": bT2,
            "msel": msel,
        })
    return in_maps


def kernel(x, kernel_base, kernel_mask, demog_label, epoch):
    nc = get_nc()
    in_maps = make_in_maps(x, kernel_base, kernel_mask, demog_label, epoch)
    res = run_bass_kernel_spmd(nc, in_maps, list(range(NCORES)))
    return np.concatenate(
        [res.results[c]["out"].astype(np.float32) for c in range(NCORES)],
        axis=0)


# revision 26
# speedup vs baseline: 1.1861x; 1.1861x over previous
"""AdaConv2d (per-sample masked 3x3 conv) on 8 TRN2 NeuronCores.

Strategy (data-parallel, per sharding hint):
  - 64 samples sharded 8-per-core; kernel_base/kernel_mask replicated.
  - Host ships, per sample, a [128, 114*114] bf16 buffer: partitions
    0-63 hold the zero-padded image (one input channel per partition),
    partitions 64-127 hold the same image shifted down one padded row.
  - Conv = 6 accumulated matmul passes per 4-row output block:
    3 "big" passes contract K=128 = (tap(0,dx) on the top half,
    tap(1,dx) on the shifted bottom half) and 3 "small" K=64 passes
    cover tap(2,dx). TRN2 streams ~2 rhs columns/cycle across the two
    64-column PE groups, so two blocks (even->PSUM parts 0-63,
    odd->parts 64-127) run concurrently.
  - Per-sample kernels = kernel_base * kernel_mask[label] computed on
    device (one broadcast tensor_tensor per sample, cast to bf16); the
    host only does layout/padding transposes and the label gather of
    mask rows.
"""
import numpy as np
import ml_dtypes

import concourse.bass as bass  # noqa: F401  (registers engines)
import concourse.tile as tile
from concourse import bacc, mybir
from concourse.bass_utils import run_bass_kernel_spmd

NCORES = 8
SPC = 8            # samples per core
H = W = 112
IC = OC = 64
ND = 4             # demographic groups
PW = H + 2         # padded width/height
PHW = PW * PW
RB = 4             # output rows per matmul block
N = RB * W         # 448 columns per matmul (one PSUM bank)
BLOCKS = H // RB   # 28 blocks per sample
ROUNDS = BLOCKS // 2
NPASS = 6          # 3 big (K=128) + 3 small (K=64)
FUSE_EPOCH = 9
F32 = mybir.dt.float32
BF16 = mybir.dt.bfloat16

# weight column layout: j=0..2 big pass dx=j (top=tap(0,dx),
# bottom=tap(1,dx)); j=3..5 small pass dx=j-3 (top=tap(2,dx))
TAP_TOP = [(0, 0), (0, 1), (0, 2), (2, 0), (2, 1), (2, 2)]
TAP_BOT = [(1, 0), (1, 1), (1, 2), (2, 0), (2, 1), (2, 2)]

_CACHE = {}


def _build():
    nc = bacc.Bacc("TRN2", target_bir_lowering=False, debug=False,
                   num_devices=NCORES)
    xs = nc.dram_tensor("xs", [SPC, 128, PHW], BF16,
                        kind="ExternalInput").ap()
    bT = nc.dram_tensor("bT", [128, NPASS * OC], F32,
                        kind="ExternalInput").ap()
    msel = nc.dram_tensor("msel", [128, SPC * NPASS], F32,
                          kind="ExternalInput").ap()
    out = nc.dram_tensor("out", [SPC, OC, H, W], BF16,
                         kind="ExternalOutput").ap()

    # [sample, round, blk, oc, rb*w]
    ov = out.rearrange("b oc (r blk rb) w -> b r blk oc (rb w)",
                       blk=2, rb=RB)

    with tile.TileContext(nc) as tc:
        with (
            tc.tile_pool(name="const", bufs=1) as constp,
            tc.tile_pool(name="xp", bufs=3) as xp,
            tc.tile_pool(name="wp", bufs=2) as wp,
            tc.tile_pool(name="stage", bufs=14) as stp,
            tc.tile_pool(name="psum", bufs=4, space="PSUM") as pp,
        ):
            bT_t = constp.tile([128, NPASS * OC], F32, name="bT_t",
                               tag="bT_t")
            nc.scalar.dma_start(bT_t[:], bT[:])
            msel_t = constp.tile([128, SPC * NPASS], F32, name="msel_t",
                                 tag="msel_t")
            nc.scalar.dma_start(msel_t[:], msel[:])
            b3 = bT_t.rearrange("p (j oc) -> p j oc", oc=OC)

            for s in range(SPC):
                xt = xp.tile([128, PHW], BF16, name="xt", tag="xt")
                # chunked loads, paced against the previous sample's
                # compute so input bursts never starve the output DMAs
                NCH = 8
                for q in range(NCH):
                    qs = (PHW // NCH) * q
                    qe = PHW if q == NCH - 1 else (PHW // NCH) * (q + 1)
                    nc.scalar.dma_start(xt[:, qs:qe], xs[s][:, qs:qe])
                x3 = xt.rearrange("p (r c) -> p r c", c=PW)

                wt = wp.tile([128, NPASS * OC], BF16, name="wt", tag="wt")
                w3 = wt.rearrange("p (j oc) -> p j oc", oc=OC)
                m3 = (msel_t[:, s * NPASS:(s + 1) * NPASS]
                      .unsqueeze(-1).broadcast_to([128, NPASS, OC]))
                nc.vector.tensor_tensor(w3[:], b3[:], m3,
                                        op=mybir.AluOpType.mult)

                for rnd in range(ROUNDS):
                    psE = pp.tile([128, N], F32, name="psE", tag="psE")
                    psO = pp.tile([128, N], F32, name="psO", tag="psO")
                    ps = (psE, psO)
                    for j in range(NPASS):
                        dx = j % 3
                        first, last = (j == 0), (j == NPASS - 1)
                        for blk in range(2):
                            r0 = (rnd * 2 + blk) * RB
                            pc = blk * 64
                            if j < 3:   # big pass: K=128, taps (0/1, dx)
                                rhs = x3[:, r0:r0 + RB, dx:dx + W]
                                lhsT = w3[:, j, :]
                            else:       # small pass: K=64, tap (2, dx)
                                rhs = x3[0:64, r0 + 2:r0 + 2 + RB,
                                         dx:dx + W]
                                lhsT = w3[0:64, j, :]
                            nc.tensor.matmul(ps[blk][pc:pc + 64, :],
                                             lhsT, rhs,
                                             start=first, stop=last)

                    st = stp.tile([128, N], BF16, name="st", tag="st")
                    nc.vector.tensor_copy(st[0:64, :], psE[0:64, :])
                    nc.scalar.copy(st[64:128, :], psO[64:128, :])
                    # per-block DMAs: 64-partition outer dim spreads the
                    # transfer across all SDMA engines
                    nc.sync.dma_start(ov[s, rnd, 0], st[0:64, :])
                    nc.sync.dma_start(ov[s, rnd, 1], st[64:128, :])

    nc.compile()
    return nc


def get_nc():
    if "nc" not in _CACHE:
        _CACHE["nc"] = _build()
    return _CACHE["nc"]


def make_in_maps(x, kernel_base, kernel_mask, demog_label, epoch):
    kb = np.asarray(kernel_base, dtype=np.float32)
    km = np.asarray(kernel_mask, dtype=np.float32)
    labels = np.asarray(demog_label).astype(np.int64)
    if int(np.asarray(epoch)) >= FUSE_EPOCH:
        labels = np.zeros_like(labels)

    B = labels.shape[0]
    # padded bf16 image + one-row-down shifted copy (layout only)
    xb = np.asarray(x, dtype=np.float32).astype(ml_dtypes.bfloat16)
    xpad = np.zeros((B, IC, PW, PW), dtype=ml_dtypes.bfloat16)
    xpad[:, :, 1:H + 1, 1:W + 1] = xb
    flat = xpad.reshape(B, IC, PHW)
    xfull = np.zeros((B, 128, PHW), dtype=ml_dtypes.bfloat16)
    xfull[:, 0:IC, :] = flat
    xfull[:, IC:, :PHW - PW] = flat[:, :, PW:]

    # bT2[p, j, oc]: top = base taps TAP_TOP, bottom = TAP_BOT (0 pad)
    kb9 = kb.reshape(OC, IC, 9)            # tap index = 3*dy + dx
    bT2 = np.zeros((128, NPASS, OC), dtype=np.float32)
    km9 = km.reshape(ND, IC, 9)
    for j, (dy, dx) in enumerate(TAP_TOP):
        bT2[0:IC, j, :] = kb9[:, :, 3 * dy + dx].T
    for j, tap in enumerate(TAP_BOT):
        if tap is not None:
            dy, dx = tap
            bT2[IC:, j, :] = kb9[:, :, 3 * dy + dx].T
    bT2 = bT2.reshape(128, NPASS * OC)

    in_maps = []
    for c in range(NCORES):
        lab = labels[c * SPC:(c + 1) * SPC]
        msel = np.zeros((128, SPC * NPASS), dtype=np.float32)
        for s in range(SPC):
            for j, (dy, dx) in enumerate(TAP_TOP):
                msel[0:IC, s * NPASS + j] = km9[lab[s], :, 3 * dy + dx]
            for j, tap in enumerate(TAP_BOT):
                if tap is not None:
                    dy, dx = tap
                    msel[IC:, s * NPASS + j] = km9[lab[s], :, 3 * dy + dx]
        in_maps.append({
            "xs": np.ascontiguousarray(xfull[c * SPC:(c + 1) * SPC]),
            "bT": bT2,
            "msel": msel,
        })
    return in_maps


def kernel(x, kernel_base, kernel_mask, demog_label, epoch):
    nc = get_nc()
    in_maps = make_in_maps(x, kernel_base, kernel_mask, demog_label, epoch)
    res = run_bass_kernel_spmd(nc, in_maps, list(range(NCORES)))
    return np.concatenate(
        [res.results[c]["out"].astype(np.float32) for c in range(NCORES)],
        axis=0)
